# revision 17
# baseline (speedup 1.0000x reference)
"""Dot-product attention (B=16, Lq=Lv=2048, D=1024) on 8 TRN2 NeuronCores.

Data-parallel over the batch dim: core i handles batch elements [2i, 2i+1].
Per batch element, per 128-row q-tile:
  S = Q @ V^T        (fp32r matmuls, contraction d on partitions)
  A = softmax(S)     (DVE row-max, ACT exp with fused row-sum, DVE scale)
  C = A @ V          (fp32r matmuls, contraction k on partitions)
Returns (context, attn) exactly like the reference module.
"""

import sys

import numpy as np

if "/opt/trn_rl_repo" not in sys.path:
    sys.path.insert(0, "/opt/trn_rl_repo")

import concourse.bass as bass
import concourse.mybir as mybir
import concourse.tile as tile
from concourse.bass_utils import run_bass_kernel_spmd
from concourse.masks import make_identity

F32 = mybir.dt.float32
F32R = mybir.dt.float32r

B, LQ, LK, D = 16, 2048, 2048, 1024
N_CORES = 8
BPC = B // N_CORES  # batch elements per core
P = 128             # SBUF/PSUM partitions
NB = 512            # one PSUM bank of fp32


def _split_multi_waits(nc):
    """This walrus build allows only one sync-wait command per instruction;
    move extra waits onto standalone EventSemaphore carriers just before."""
    for f in nc.m.functions:
        for blk in f.blocks:
            out = []
            for inst in blk.instructions:
                si = getattr(inst, "sync_info", None)
                if si is not None and si.on_wait is not None and len(si.on_wait) > 1:
                    waits = list(si.on_wait)
                    for w in waits[:-1]:
                        nop = mybir.InstEventSemaphore(
                            name=f"I-{nc.next_id()}", ins=[], outs=[]
                        )
                        nop.engine = inst.engine
                        nop.sync_info = mybir.SyncInfo(on_wait=[w], on_update=[])
                        out.append(nop)
                    inst.sync_info = mybir.SyncInfo(
                        on_wait=[waits[-1]], on_update=list(si.on_update)
                    )
                out.append(inst)
            blk.instructions = out


def build_nc(bpc=BPC, lq=LQ, lk=LK, d=D, mm_dtype=F32R, split_waits=True):
    """Build + compile the single-core Bass program (same program on all cores)."""
    n_qt = lq // P     # q row-blocks per batch element
    n_kc = lk // P     # k chunks (contraction tiles for C; width tiles for S)
    n_dc = d // P      # d chunks (contraction tiles for S)
    n_sb = lk // NB    # PSUM banks per S row-block
    n_cb = d // NB     # PSUM banks per C row-block

    nc = bass.Bass()
    q_d = nc.dram_tensor("query", [bpc, lq, d], F32, kind="ExternalInput")
    v_d = nc.dram_tensor("value", [bpc, lk, d], F32, kind="ExternalInput")
    ctx_d = nc.dram_tensor("context", [bpc, lq, d], F32, kind="ExternalOutput")
    attn_d = nc.dram_tensor("attn", [bpc, lq, lk], F32, kind="ExternalOutput")

    # Tiles consumed by reduced-precision matmuls carry mm_dtype themselves
    # (the BIR verifier requires fp32r consumers to read fp32r-rounded data),
    # so producers (ACT copies / DMA) round on write.
    def mmcast(ap):
        return ap.bitcast(mm_dtype) if mm_dtype != F32 else ap

    with tile.TileContext(nc) as tc:
        with (
            tc.tile_pool(name="const", bufs=1) as constp,
            tc.tile_pool(name="vres", bufs=1) as vres,
            tc.tile_pool(name="qload", bufs=2) as qload,
            tc.tile_pool(name="qt", bufs=2) as qtp,
            tc.tile_pool(name="e", bufs=2) as ep,
            tc.tile_pool(name="et", bufs=2) as etp,
            tc.tile_pool(name="c", bufs=2) as cp,
            tc.tile_pool(name="stats", bufs=6) as statp,
            tc.tile_pool(name="s_ps", bufs=1, space=bass.MemorySpace.PSUM) as spsp,
            tc.tile_pool(name="c_ps", bufs=1, space=bass.MemorySpace.PSUM) as cpsp,
            tc.tile_pool(name="t_ps", bufs=2, space=bass.MemorySpace.PSUM) as tpsp,
        ):
            ident = constp.tile([P, P], F32, tag="ident")
            make_identity(nc, ident[:])
            if mm_dtype != F32:
                ident_r = constp.tile([P, P], mm_dtype, tag="ident_r")
                nc.scalar.copy(ident_r[:], ident[:])
            else:
                ident_r = ident

            for b in range(bpc):
                # V resident in both layouts.
                # v_sb[p, kc, :]   = V[kc*P + p, :]            (natural)
                # vt_sb[p, dc, k]  = V[k, dc*P + p]            (transposed)
                v_sb = vres.tile([P, n_kc, d], mm_dtype, tag="v")
                vt_sb = vres.tile([P, n_dc, lk], mm_dtype, tag="vt")
                for kc in range(n_kc):
                    nc.sync.dma_start(
                        out=v_sb[:, kc, :],
                        in_=mmcast(v_d[b, kc * P:(kc + 1) * P, :]),
                    )
                for kc in range(n_kc):
                    for g in range(n_dc // 4):
                        t = tpsp.tile([P, 4, P], mm_dtype, tag="tps")
                        for j in range(4):
                            dc = g * 4 + j
                            nc.tensor.transpose(
                                t[:, j, :],
                                v_sb[:, kc, dc * P:(dc + 1) * P],
                                ident_r[:],
                            )
                        nc.scalar.copy(
                            vt_sb[:, g * 4:(g + 1) * 4, kc * P:(kc + 1) * P], t[:]
                        )

                for qi in range(n_qt):
                    q0 = qi * P
                    # Q row-block, then Q^T tiles: qt_sb[p, dc, f] = Q[q0+f, dc*P+p]
                    q_nat = qload.tile([P, d], F32, tag="qnat")
                    nc.sync.dma_start(out=q_nat[:], in_=q_d[b, q0:q0 + P, :])
                    qt_sb = qtp.tile([P, n_dc, P], mm_dtype, tag="qt")
                    for g in range(n_dc // 4):
                        t = tpsp.tile([P, 4, P], F32, tag="tps")
                        for j in range(4):
                            dc = g * 4 + j
                            nc.tensor.transpose(
                                t[:, j, :], q_nat[:, dc * P:(dc + 1) * P], ident[:]
                            )
                        nc.scalar.copy(qt_sb[:, g * 4:(g + 1) * 4, :], t[:])

                    # S[q, k] = sum_d Q[q, d] V[k, d]
                    s_ps = spsp.tile([P, lk], F32, tag="sps")
                    for dc in range(n_dc):
                        for n in range(n_sb):
                            nc.tensor.matmul(
                                s_ps[:, n * NB:(n + 1) * NB],
                                qt_sb[:, dc, :],
                                vt_sb[:, dc, n * NB:(n + 1) * NB],
                                start=(dc == 0),
                                stop=(dc == n_dc - 1),
                            )

                    # softmax over k: E = exp(S - rowmax), ssum = row sums
                    negm = statp.tile([P, 1], F32, tag="negm")
                    nc.vector.reduce_max(
                        out=negm[:], in_=s_ps[:], axis=mybir.AxisListType.X, negate=True
                    )
                    e_sb = ep.tile([P, lk], F32, tag="e")
                    ssum = statp.tile([P, 1], F32, tag="ssum")
                    nc.scalar.activation(
                        e_sb[:], s_ps[:], mybir.ActivationFunctionType.Exp,
                        bias=negm[:], scale=1.0, accum_out=ssum[:],
                    )
                    rinv = statp.tile([P, 1], F32, tag="rinv")
                    nc.vector.reciprocal(rinv[:], ssum[:])

                    # E^T tiles: et_sb[p, kc, f] = E[f, kc*P+p]
                    et_sb = etp.tile([P, n_kc, P], mm_dtype, tag="et")
                    for g in range(n_kc // 4):
                        t = tpsp.tile([P, 4, P], F32, tag="tps")
                        for j in range(4):
                            kc = g * 4 + j
                            nc.tensor.transpose(
                                t[:, j, :], e_sb[:, kc * P:(kc + 1) * P], ident[:]
                            )
                        nc.scalar.copy(et_sb[:, g * 4:(g + 1) * 4, :], t[:])

                    # C[q, d] = sum_k E[q, k] V[k, d]  (normalized at copy-out)
                    c_ps = cpsp.tile([P, d], F32, tag="cps")
                    for kc in range(n_kc):
                        for dh in range(n_cb):
                            nc.tensor.matmul(
                                c_ps[:, dh * NB:(dh + 1) * NB],
                                et_sb[:, kc, :],
                                v_sb[:, kc, dh * NB:(dh + 1) * NB],
                                start=(kc == 0),
                                stop=(kc == n_kc - 1),
                            )

                    # attn row-block: A = E * (1/ssum), in place, then store
                    nc.vector.tensor_scalar_mul(e_sb[:], e_sb[:], rinv[:])
                    nc.sync.dma_start(out=attn_d[b, q0:q0 + P, :], in_=e_sb[:])

                    # context row-block: C = c_ps * (1/ssum), then store
                    c_sb = cp.tile([P, d], F32, tag="c")
                    nc.scalar.mul(c_sb[:], c_ps[:], rinv[:])
                    nc.sync.dma_start(out=ctx_d[b, q0:q0 + P, :], in_=c_sb[:])

    if split_waits:
        _split_multi_waits(nc)
    nc.finalize()
    return nc


_CACHE: dict = {}


def _get_nc():
    if "nc" not in _CACHE:
        _CACHE["nc"] = build_nc()
    return _CACHE["nc"]


def make_in_maps(query, value):
    query = np.ascontiguousarray(np.asarray(query, dtype=np.float32))
    value = np.ascontiguousarray(np.asarray(value, dtype=np.float32))
    return [
        {
            "query": query[i * BPC:(i + 1) * BPC],
            "value": value[i * BPC:(i + 1) * BPC],
        }
        for i in range(N_CORES)
    ]


def kernel(query, value):
    nc = _get_nc()
    res = run_bass_kernel_spmd(
        nc, make_in_maps(query, value), core_ids=list(range(N_CORES))
    ).results
    context = np.concatenate([r["context"] for r in res], axis=0)
    attn = np.concatenate([r["attn"] for r in res], axis=0)
    return context, attn


# revision 20
# speedup vs baseline: 1.0660x; 1.0660x over previous
"""Dot-product attention (B=16, Lq=Lv=2048, D=1024) on 8 TRN2 NeuronCores.

Data-parallel over the batch dim: core i handles batch elements [2i, 2i+1].
Per batch element, per 128-row q-tile:
  S = Q @ V^T        (fp32r matmuls, contraction d on partitions)
  A = softmax(S)     (DVE row-max, ACT exp with fused row-sum, DVE scale)
  C = A @ V          (fp32r matmuls, contraction k on partitions)
Returns (context, attn) exactly like the reference module.
"""

import sys

import numpy as np

if "/opt/trn_rl_repo" not in sys.path:
    sys.path.insert(0, "/opt/trn_rl_repo")

import concourse.bass as bass
import concourse.mybir as mybir
import concourse.tile as tile
from concourse.bass_utils import run_bass_kernel_spmd
from concourse.masks import make_identity

F32 = mybir.dt.float32
F32R = mybir.dt.float32r

B, LQ, LK, D = 16, 2048, 2048, 1024
N_CORES = 8
BPC = B // N_CORES  # batch elements per core
P = 128             # SBUF/PSUM partitions
NB = 512            # one PSUM bank of fp32


def _split_multi_waits(nc):
    """This walrus build allows only one sync-wait command per instruction;
    move extra waits onto standalone EventSemaphore carriers just before."""
    for f in nc.m.functions:
        for blk in f.blocks:
            out = []
            for inst in blk.instructions:
                si = getattr(inst, "sync_info", None)
                if si is not None and si.on_wait is not None and len(si.on_wait) > 1:
                    waits = list(si.on_wait)
                    for w in waits[:-1]:
                        nop = mybir.InstEventSemaphore(
                            name=f"I-{nc.next_id()}", ins=[], outs=[]
                        )
                        nop.engine = inst.engine
                        nop.sync_info = mybir.SyncInfo(on_wait=[w], on_update=[])
                        out.append(nop)
                    inst.sync_info = mybir.SyncInfo(
                        on_wait=[waits[-1]], on_update=list(si.on_update)
                    )
                out.append(inst)
            blk.instructions = out


def build_nc(bpc=BPC, lq=LQ, lk=LK, d=D, mm_dtype=F32R, split_waits=True):
    """Build + compile the single-core Bass program (same program on all cores)."""
    n_qt = lq // P     # q row-blocks per batch element
    n_kc = lk // P     # k chunks (contraction tiles for C; width tiles for S)
    n_dc = d // P      # d chunks (contraction tiles for S)
    n_sb = lk // NB    # PSUM banks per S row-block
    n_cb = d // NB     # PSUM banks per C row-block

    nc = bass.Bass()
    q_d = nc.dram_tensor("query", [bpc, lq, d], F32, kind="ExternalInput")
    v_d = nc.dram_tensor("value", [bpc, lk, d], F32, kind="ExternalInput")
    ctx_d = nc.dram_tensor("context", [bpc, lq, d], F32, kind="ExternalOutput")
    attn_d = nc.dram_tensor("attn", [bpc, lq, lk], F32, kind="ExternalOutput")

    # Tiles consumed by reduced-precision matmuls carry mm_dtype themselves
    # (the BIR verifier requires fp32r consumers to read fp32r-rounded data),
    # so producers (ACT copies / DMA) round on write.
    def mmcast(ap):
        return ap.bitcast(mm_dtype) if mm_dtype != F32 else ap

    with tile.TileContext(nc) as tc:
        with (
            tc.tile_pool(name="const", bufs=1) as constp,
            tc.tile_pool(name="vres", bufs=1) as vres,
            tc.tile_pool(name="qload", bufs=2) as qload,
            tc.tile_pool(name="qt", bufs=2) as qtp,
            tc.tile_pool(name="e", bufs=2) as ep,
            tc.tile_pool(name="et", bufs=2) as etp,
            tc.tile_pool(name="c", bufs=2) as cp,
            tc.tile_pool(name="stats", bufs=3) as statp,
            tc.tile_pool(name="s_ps", bufs=4, space=bass.MemorySpace.PSUM) as spsp,
            tc.tile_pool(name="c_ps", bufs=1, space=bass.MemorySpace.PSUM) as cpsp,
            tc.tile_pool(name="t_ps", bufs=2, space=bass.MemorySpace.PSUM) as tpsp,
        ):
            ident = constp.tile([P, P], F32, tag="ident")
            make_identity(nc, ident[:])
            if mm_dtype != F32:
                ident_r = constp.tile([P, P], mm_dtype, tag="ident_r")
                nc.scalar.copy(ident_r[:], ident[:])
            else:
                ident_r = ident

            for b in range(bpc):
                # V resident in both layouts.
                # v_sb[p, kc, :]   = V[kc*P + p, :]            (natural)
                # vt_sb[p, dc, k]  = V[k, dc*P + p]            (transposed)
                v_sb = vres.tile([P, n_kc, d], mm_dtype, tag="v")
                vt_sb = vres.tile([P, n_dc, lk], mm_dtype, tag="vt")
                for kc in range(n_kc):
                    nc.sync.dma_start(
                        out=v_sb[:, kc, :],
                        in_=mmcast(v_d[b, kc * P:(kc + 1) * P, :]),
                    )
                for kc in range(n_kc):
                    for g in range(n_dc // 4):
                        t = tpsp.tile([P, 4, P], mm_dtype, tag="tps")
                        for j in range(4):
                            dc = g * 4 + j
                            nc.tensor.transpose(
                                t[:, j, :],
                                v_sb[:, kc, dc * P:(dc + 1) * P],
                                ident_r[:],
                            )
                        nc.scalar.copy(
                            vt_sb[:, g * 4:(g + 1) * 4, kc * P:(kc + 1) * P], t[:]
                        )

                for qi in range(n_qt):
                    q0 = qi * P
                    # Q row-block, then Q^T tiles: qt_sb[p, dc, f] = Q[q0+f, dc*P+p]
                    q_nat = qload.tile([P, d], F32, tag="qnat")
                    nc.sync.dma_start(out=q_nat[:], in_=q_d[b, q0:q0 + P, :])
                    qt_sb = qtp.tile([P, n_dc, P], mm_dtype, tag="qt")
                    for g in range(n_dc // 4):
                        t = tpsp.tile([P, 4, P], F32, tag="tps")
                        for j in range(4):
                            dc = g * 4 + j
                            nc.tensor.transpose(
                                t[:, j, :], q_nat[:, dc * P:(dc + 1) * P], ident[:]
                            )
                        nc.scalar.copy(qt_sb[:, g * 4:(g + 1) * 4, :], t[:])

                    # S[q, k] = sum_d Q[q, d] V[k, d], one PSUM bank per 512-wide
                    # chunk; per-chunk max+exp frees each bank early so the
                    # next tile's S matmuls start without waiting for softmax.
                    e_sb = ep.tile([P, lk], F32, tag="e")
                    negmax = statp.tile([P, n_sb], F32, tag="negmax")
                    csum = statp.tile([P, n_sb], F32, tag="csum")
                    for n in range(n_sb):
                        s_ch = spsp.tile([P, NB], F32, tag="sch")
                        for dc in range(n_dc):
                            nc.tensor.matmul(
                                s_ch[:],
                                qt_sb[:, dc, :],
                                vt_sb[:, dc, n * NB:(n + 1) * NB],
                                start=(dc == 0),
                                stop=(dc == n_dc - 1),
                            )
                        nc.vector.reduce_max(
                            out=negmax[:, n:n + 1], in_=s_ch[:],
                            axis=mybir.AxisListType.X, negate=True,
                        )
                        nc.scalar.activation(
                            e_sb[:, n * NB:(n + 1) * NB], s_ch[:],
                            mybir.ActivationFunctionType.Exp,
                            bias=negmax[:, n:n + 1], scale=1.0,
                            accum_out=csum[:, n:n + 1],
                        )

                    # combine chunks exactly: f_n = exp(max_n - M),
                    # total = sum_n f_n * csum_n, g_n = f_n / total
                    negM = statp.tile([P, 1], F32, tag="negM")
                    nc.vector.tensor_reduce(
                        out=negM[:], in_=negmax[:], axis=mybir.AxisListType.X,
                        op=mybir.AluOpType.min,
                    )
                    f = statp.tile([P, n_sb], F32, tag="f")
                    nc.scalar.activation(
                        f[:], negmax[:], mybir.ActivationFunctionType.Exp,
                        bias=negM[:], scale=-1.0,
                    )
                    fc = statp.tile([P, n_sb], F32, tag="fc")
                    stot = statp.tile([P, 1], F32, tag="stot")
                    nc.vector.tensor_mul(fc[:], f[:], csum[:])
                    nc.vector.reduce_sum(
                        out=stot[:], in_=fc[:], axis=mybir.AxisListType.X
                    )
                    rinv = statp.tile([P, 1], F32, tag="rinv")
                    nc.vector.reciprocal(rinv[:], stot[:])
                    g = statp.tile([P, n_sb], F32, tag="g")
                    nc.vector.tensor_scalar_mul(g[:], f[:], rinv[:])

                    # normalize E in place -> final attn row-block
                    for n in range(n_sb):
                        nc.vector.tensor_scalar_mul(
                            e_sb[:, n * NB:(n + 1) * NB],
                            e_sb[:, n * NB:(n + 1) * NB],
                            g[:, n:n + 1],
                        )

                    # A^T tiles: et_sb[p, kc, f] = A[f, kc*P+p]
                    et_sb = etp.tile([P, n_kc, P], mm_dtype, tag="et")
                    for gi in range(n_kc // 4):
                        t = tpsp.tile([P, 4, P], F32, tag="tps")
                        for j in range(4):
                            kc = gi * 4 + j
                            nc.tensor.transpose(
                                t[:, j, :], e_sb[:, kc * P:(kc + 1) * P], ident[:]
                            )
                        nc.scalar.copy(et_sb[:, gi * 4:(gi + 1) * 4, :], t[:])

                    nc.sync.dma_start(out=attn_d[b, q0:q0 + P, :], in_=e_sb[:])

                    # C[q, d] = sum_k A[q, k] V[k, d]
                    c_ps = cpsp.tile([P, d], F32, tag="cps")
                    for kc in range(n_kc):
                        for dh in range(n_cb):
                            nc.tensor.matmul(
                                c_ps[:, dh * NB:(dh + 1) * NB],
                                et_sb[:, kc, :],
                                v_sb[:, kc, dh * NB:(dh + 1) * NB],
                                start=(kc == 0),
                                stop=(kc == n_kc - 1),
                            )
                    c_sb = cp.tile([P, d], F32, tag="c")
                    nc.scalar.copy(c_sb[:], c_ps[:])
                    nc.sync.dma_start(out=ctx_d[b, q0:q0 + P, :], in_=c_sb[:])

    if split_waits:
        _split_multi_waits(nc)
    nc.finalize()
    return nc


_CACHE: dict = {}


def _get_nc():
    if "nc" not in _CACHE:
        _CACHE["nc"] = build_nc()
    return _CACHE["nc"]


def make_in_maps(query, value):
    query = np.ascontiguousarray(np.asarray(query, dtype=np.float32))
    value = np.ascontiguousarray(np.asarray(value, dtype=np.float32))
    return [
        {
            "query": query[i * BPC:(i + 1) * BPC],
            "value": value[i * BPC:(i + 1) * BPC],
        }
        for i in range(N_CORES)
    ]


def kernel(query, value):
    nc = _get_nc()
    res = run_bass_kernel_spmd(
        nc, make_in_maps(query, value), core_ids=list(range(N_CORES))
    ).results
    context = np.concatenate([r["context"] for r in res], axis=0)
    attn = np.concatenate([r["attn"] for r in res], axis=0)
    return context, attn


# revision 22
# speedup vs baseline: 1.2990x; 1.2185x over previous
"""Dot-product attention (B=16, Lq=Lv=2048, D=1024) on 8 TRN2 NeuronCores.

Data-parallel over the batch dim: core i handles batch elements [2i, 2i+1].
Per batch element, per 128-row q-tile:
  S = Q @ V^T        (fp32r matmuls, contraction d on partitions)
  A = softmax(S)     (DVE row-max, ACT exp with fused row-sum, DVE scale)
  C = A @ V          (fp32r matmuls, contraction k on partitions)
Returns (context, attn) exactly like the reference module.
"""

import sys

import numpy as np

if "/opt/trn_rl_repo" not in sys.path:
    sys.path.insert(0, "/opt/trn_rl_repo")

import concourse.bass as bass
import concourse.mybir as mybir
import concourse.tile as tile
from concourse.bass_utils import run_bass_kernel_spmd
from concourse.masks import make_identity

F32 = mybir.dt.float32
F32R = mybir.dt.float32r

B, LQ, LK, D = 16, 2048, 2048, 1024
N_CORES = 8
BPC = B // N_CORES  # batch elements per core
P = 128             # SBUF/PSUM partitions
NB = 512            # one PSUM bank of fp32


def _split_multi_waits(nc):
    """This walrus build allows only one sync-wait command per instruction;
    move extra waits onto standalone EventSemaphore carriers just before."""
    for f in nc.m.functions:
        for blk in f.blocks:
            out = []
            for inst in blk.instructions:
                si = getattr(inst, "sync_info", None)
                if si is not None and si.on_wait is not None and len(si.on_wait) > 1:
                    waits = list(si.on_wait)
                    for w in waits[:-1]:
                        nop = mybir.InstEventSemaphore(
                            name=f"I-{nc.next_id()}", ins=[], outs=[]
                        )
                        nop.engine = inst.engine
                        nop.sync_info = mybir.SyncInfo(on_wait=[w], on_update=[])
                        out.append(nop)
                    inst.sync_info = mybir.SyncInfo(
                        on_wait=[waits[-1]], on_update=list(si.on_update)
                    )
                out.append(inst)
            blk.instructions = out


def build_nc(bpc=BPC, lq=LQ, lk=LK, d=D, mm_dtype=F32R, split_waits=True):
    """Build + compile the single-core Bass program (same program on all cores)."""
    n_qt = lq // P     # q row-blocks per batch element
    n_kc = lk // P     # k chunks (contraction tiles for C; width tiles for S)
    n_dc = d // P      # d chunks (contraction tiles for S)
    n_sb = lk // NB    # PSUM banks per S row-block
    n_cb = d // NB     # PSUM banks per C row-block

    nc = bass.Bass()
    q_d = nc.dram_tensor("query", [bpc, lq, d], F32, kind="ExternalInput")
    v_d = nc.dram_tensor("value", [bpc, lk, d], F32, kind="ExternalInput")
    ctx_d = nc.dram_tensor("context", [bpc, lq, d], F32, kind="ExternalOutput")
    attn_d = nc.dram_tensor("attn", [bpc, lq, lk], F32, kind="ExternalOutput")

    # Tiles consumed by reduced-precision matmuls carry mm_dtype themselves
    # (the BIR verifier requires fp32r consumers to read fp32r-rounded data),
    # so producers (ACT copies / DMA) round on write.
    def mmcast(ap):
        return ap.bitcast(mm_dtype) if mm_dtype != F32 else ap

    with tile.TileContext(nc) as tc:
        with (
            tc.tile_pool(name="const", bufs=1) as constp,
            tc.tile_pool(name="vres", bufs=1) as vres,
            tc.tile_pool(name="qload", bufs=2) as qload,
            tc.tile_pool(name="qt", bufs=2) as qtp,
            tc.tile_pool(name="e", bufs=2) as ep,
            tc.tile_pool(name="et", bufs=2) as etp,
            tc.tile_pool(name="c", bufs=2) as cp,
            tc.tile_pool(name="stats", bufs=3) as statp,
            tc.tile_pool(name="s_ps", bufs=4, space=bass.MemorySpace.PSUM) as spsp,
            tc.tile_pool(name="c_ps", bufs=1, space=bass.MemorySpace.PSUM) as cpsp,
            tc.tile_pool(name="t_ps", bufs=2, space=bass.MemorySpace.PSUM) as tpsp,
        ):
            ident = constp.tile([P, P], F32, tag="ident")
            make_identity(nc, ident[:])
            if mm_dtype != F32:
                ident_r = constp.tile([P, P], mm_dtype, tag="ident_r")
                nc.scalar.copy(ident_r[:], ident[:])
            else:
                ident_r = ident

            # Software pipeline: each q-tile's "tail" (A^T transposes + C
            # matmuls) is emitted after the NEXT tile's "head" (Q^T + S
            # matmuls).  Engine streams execute in program order, so this
            # gives the PE independent S-work to chew on while the softmax
            # combine chain (DVE/ACT) of the previous tile completes.
            pending_tail = None

            for b in range(bpc):
                if pending_tail is not None:
                    pending_tail()
                    pending_tail = None
                # V resident in both layouts.
                # v_sb[p, kc, :]   = V[kc*P + p, :]            (natural)
                # vt_sb[p, dc, k]  = V[k, dc*P + p]            (transposed)
                v_sb = vres.tile([P, n_kc, d], mm_dtype, tag="v")
                vt_sb = vres.tile([P, n_dc, lk], mm_dtype, tag="vt")
                for kc in range(n_kc):
                    nc.sync.dma_start(
                        out=v_sb[:, kc, :],
                        in_=mmcast(v_d[b, kc * P:(kc + 1) * P, :]),
                    )
                for kc in range(n_kc):
                    for g in range(n_dc // 4):
                        t = tpsp.tile([P, 4, P], mm_dtype, tag="tps")
                        for j in range(4):
                            dc = g * 4 + j
                            nc.tensor.transpose(
                                t[:, j, :],
                                v_sb[:, kc, dc * P:(dc + 1) * P],
                                ident_r[:],
                            )
                        nc.scalar.copy(
                            vt_sb[:, g * 4:(g + 1) * 4, kc * P:(kc + 1) * P], t[:]
                        )

                for qi in range(n_qt):
                    q0 = qi * P
                    # Q row-block, then Q^T tiles: qt_sb[p, dc, f] = Q[q0+f, dc*P+p]
                    q_nat = qload.tile([P, d], F32, tag="qnat")
                    nc.sync.dma_start(out=q_nat[:], in_=q_d[b, q0:q0 + P, :])
                    qt_sb = qtp.tile([P, n_dc, P], mm_dtype, tag="qt")
                    for g in range(n_dc // 4):
                        t = tpsp.tile([P, 4, P], F32, tag="tps")
                        for j in range(4):
                            dc = g * 4 + j
                            nc.tensor.transpose(
                                t[:, j, :], q_nat[:, dc * P:(dc + 1) * P], ident[:]
                            )
                        nc.scalar.copy(qt_sb[:, g * 4:(g + 1) * 4, :], t[:])

                    # S[q, k] = sum_d Q[q, d] V[k, d], one PSUM bank per 512-wide
                    # chunk; per-chunk max+exp frees each bank early so the
                    # next tile's S matmuls start without waiting for softmax.
                    e_sb = ep.tile([P, lk], F32, tag="e")
                    negmax = statp.tile([P, n_sb], F32, tag="negmax")
                    csum = statp.tile([P, n_sb], F32, tag="csum")
                    for n in range(n_sb):
                        s_ch = spsp.tile([P, NB], F32, tag="sch")
                        for dc in range(n_dc):
                            nc.tensor.matmul(
                                s_ch[:],
                                qt_sb[:, dc, :],
                                vt_sb[:, dc, n * NB:(n + 1) * NB],
                                start=(dc == 0),
                                stop=(dc == n_dc - 1),
                            )
                        nc.vector.reduce_max(
                            out=negmax[:, n:n + 1], in_=s_ch[:],
                            axis=mybir.AxisListType.X, negate=True,
                        )
                        nc.scalar.activation(
                            e_sb[:, n * NB:(n + 1) * NB], s_ch[:],
                            mybir.ActivationFunctionType.Exp,
                            bias=negmax[:, n:n + 1], scale=1.0,
                            accum_out=csum[:, n:n + 1],
                        )

                    # combine chunks exactly: f_n = exp(max_n - M),
                    # total = sum_n f_n * csum_n, g_n = f_n / total
                    negM = statp.tile([P, 1], F32, tag="negM")
                    nc.vector.tensor_reduce(
                        out=negM[:], in_=negmax[:], axis=mybir.AxisListType.X,
                        op=mybir.AluOpType.min,
                    )
                    f = statp.tile([P, n_sb], F32, tag="f")
                    nc.scalar.activation(
                        f[:], negmax[:], mybir.ActivationFunctionType.Exp,
                        bias=negM[:], scale=-1.0,
                    )
                    fc = statp.tile([P, n_sb], F32, tag="fc")
                    stot = statp.tile([P, 1], F32, tag="stot")
                    nc.vector.tensor_mul(fc[:], f[:], csum[:])
                    nc.vector.reduce_sum(
                        out=stot[:], in_=fc[:], axis=mybir.AxisListType.X
                    )
                    rinv = statp.tile([P, 1], F32, tag="rinv")
                    nc.vector.reciprocal(rinv[:], stot[:])
                    g = statp.tile([P, n_sb], F32, tag="g")
                    nc.vector.tensor_scalar_mul(g[:], f[:], rinv[:])

                    # normalize E in place -> final attn row-block
                    for n in range(n_sb):
                        nc.vector.tensor_scalar_mul(
                            e_sb[:, n * NB:(n + 1) * NB],
                            e_sb[:, n * NB:(n + 1) * NB],
                            g[:, n:n + 1],
                        )
                    nc.sync.dma_start(out=attn_d[b, q0:q0 + P, :], in_=e_sb[:])

                    def tail(b=b, q0=q0, e_sb=e_sb, v_sb=v_sb):
                        # A^T tiles: et_sb[p, kc, f] = A[f, kc*P+p]
                        et_sb = etp.tile([P, n_kc, P], mm_dtype, tag="et")
                        for gi in range(n_kc // 4):
                            t = tpsp.tile([P, 4, P], F32, tag="tps")
                            for j in range(4):
                                kc = gi * 4 + j
                                nc.tensor.transpose(
                                    t[:, j, :], e_sb[:, kc * P:(kc + 1) * P],
                                    ident[:],
                                )
                            nc.scalar.copy(et_sb[:, gi * 4:(gi + 1) * 4, :], t[:])

                        # C[q, d] = sum_k A[q, k] V[k, d]
                        c_ps = cpsp.tile([P, d], F32, tag="cps")
                        for kc in range(n_kc):
                            for dh in range(n_cb):
                                nc.tensor.matmul(
                                    c_ps[:, dh * NB:(dh + 1) * NB],
                                    et_sb[:, kc, :],
                                    v_sb[:, kc, dh * NB:(dh + 1) * NB],
                                    start=(kc == 0),
                                    stop=(kc == n_kc - 1),
                                )
                        c_sb = cp.tile([P, d], F32, tag="c")
                        nc.scalar.copy(c_sb[:], c_ps[:])
                        nc.sync.dma_start(out=ctx_d[b, q0:q0 + P, :], in_=c_sb[:])

                    if pending_tail is not None:
                        pending_tail()
                    pending_tail = tail

            if pending_tail is not None:
                pending_tail()
                pending_tail = None

    if split_waits:
        _split_multi_waits(nc)
    nc.finalize()
    return nc


_CACHE: dict = {}


def _get_nc():
    if "nc" not in _CACHE:
        _CACHE["nc"] = build_nc()
    return _CACHE["nc"]


def make_in_maps(query, value):
    query = np.ascontiguousarray(np.asarray(query, dtype=np.float32))
    value = np.ascontiguousarray(np.asarray(value, dtype=np.float32))
    return [
        {
            "query": query[i * BPC:(i + 1) * BPC],
            "value": value[i * BPC:(i + 1) * BPC],
        }
        for i in range(N_CORES)
    ]


def kernel(query, value):
    nc = _get_nc()
    res = run_bass_kernel_spmd(
        nc, make_in_maps(query, value), core_ids=list(range(N_CORES))
    ).results
    context = np.concatenate([r["context"] for r in res], axis=0)
    attn = np.concatenate([r["attn"] for r in res], axis=0)
    return context, attn


# revision 25
# speedup vs baseline: 1.3940x; 1.0732x over previous
"""Dot-product attention (B=16, Lq=Lv=2048, D=1024) on 8 TRN2 NeuronCores.

Data-parallel over the batch dim: core i handles batch elements [2i, 2i+1].
Per batch element, per 128-row q-tile:
  S = Q @ V^T        (fp32r matmuls, contraction d on partitions)
  A = softmax(S)     (DVE row-max, ACT exp with fused row-sum, DVE scale)
  C = A @ V          (fp32r matmuls, contraction k on partitions)
Returns (context, attn) exactly like the reference module.
"""

import sys

import numpy as np

if "/opt/trn_rl_repo" not in sys.path:
    sys.path.insert(0, "/opt/trn_rl_repo")

import concourse.bass as bass
import concourse.mybir as mybir
import concourse.tile as tile
from concourse.bass_utils import run_bass_kernel_spmd
from concourse.masks import make_identity

F32 = mybir.dt.float32
F32R = mybir.dt.float32r

B, LQ, LK, D = 16, 2048, 2048, 1024
N_CORES = 8
BPC = B // N_CORES  # batch elements per core
P = 128             # SBUF/PSUM partitions
NB = 512            # one PSUM bank of fp32


def _split_multi_waits(nc):
    """This walrus build allows only one sync-wait command per instruction;
    move extra waits onto standalone EventSemaphore carriers just before."""
    for f in nc.m.functions:
        for blk in f.blocks:
            out = []
            for inst in blk.instructions:
                si = getattr(inst, "sync_info", None)
                if si is not None and si.on_wait is not None and len(si.on_wait) > 1:
                    waits = list(si.on_wait)
                    for w in waits[:-1]:
                        nop = mybir.InstEventSemaphore(
                            name=f"I-{nc.next_id()}", ins=[], outs=[]
                        )
                        nop.engine = inst.engine
                        nop.sync_info = mybir.SyncInfo(on_wait=[w], on_update=[])
                        out.append(nop)
                    inst.sync_info = mybir.SyncInfo(
                        on_wait=[waits[-1]], on_update=list(si.on_update)
                    )
                out.append(inst)
            blk.instructions = out


def build_nc(bpc=BPC, lq=LQ, lk=LK, d=D, mm_dtype=F32R, split_waits=True):
    """Build + compile the single-core Bass program (same program on all cores)."""
    n_qt = lq // P     # q row-blocks per batch element
    n_kc = lk // P     # k chunks (contraction tiles for C; width tiles for S)
    n_dc = d // P      # d chunks (contraction tiles for S)
    n_sb = lk // NB    # PSUM banks per S row-block
    n_cb = d // NB     # PSUM banks per C row-block

    nc = bass.Bass()
    q_d = nc.dram_tensor("query", [bpc, lq, d], F32, kind="ExternalInput")
    v_d = nc.dram_tensor("value", [bpc, lk, d], F32, kind="ExternalInput")
    ctx_d = nc.dram_tensor("context", [bpc, lq, d], F32, kind="ExternalOutput")
    attn_d = nc.dram_tensor("attn", [bpc, lq, lk], F32, kind="ExternalOutput")

    # Tiles consumed by reduced-precision matmuls carry mm_dtype themselves
    # (the BIR verifier requires fp32r consumers to read fp32r-rounded data),
    # so producers (ACT copies / DMA) round on write.
    def mmcast(ap):
        return ap.bitcast(mm_dtype) if mm_dtype != F32 else ap

    with tile.TileContext(nc) as tc:
        with (
            tc.tile_pool(name="const", bufs=1) as constp,
            tc.tile_pool(name="vres", bufs=1) as vres,
            tc.tile_pool(name="qload", bufs=2) as qload,
            tc.tile_pool(name="qt", bufs=2) as qtp,
            tc.tile_pool(name="e", bufs=2) as ep,
            tc.tile_pool(name="et", bufs=2) as etp,
            tc.tile_pool(name="c", bufs=2) as cp,
            tc.tile_pool(name="stats", bufs=3) as statp,
            tc.tile_pool(name="s_ps", bufs=4, space=bass.MemorySpace.PSUM) as spsp,
            tc.tile_pool(name="c_ps", bufs=1, space=bass.MemorySpace.PSUM) as cpsp,
            tc.tile_pool(name="t_ps", bufs=2, space=bass.MemorySpace.PSUM) as tpsp,
        ):
            ident = constp.tile([P, P], F32, tag="ident")
            make_identity(nc, ident[:])
            if mm_dtype != F32:
                ident_r = constp.tile([P, P], mm_dtype, tag="ident_r")
                nc.scalar.copy(ident_r[:], ident[:])
            else:
                ident_r = ident

            # Software pipeline: each q-tile's "tail" (A^T transposes + C
            # matmuls) is emitted after the NEXT tile's "head" (Q^T + S
            # matmuls).  Engine streams execute in program order, so this
            # gives the PE independent S-work to chew on while the softmax
            # combine chain (DVE/ACT) of the previous tile completes.
            pending_tail = None

            for b in range(bpc):
                if pending_tail is not None:
                    pending_tail()
                    pending_tail = None
                # V resident in both layouts.
                # v_sb[p, kc, :]   = V[kc*P + p, :]            (natural)
                # vt_sb[p, dc, k]  = V[k, dc*P + p]            (transposed)
                v_sb = vres.tile([P, n_kc, d], mm_dtype, tag="v")
                vt_sb = vres.tile([P, n_dc, lk], mm_dtype, tag="vt")
                for kc in range(n_kc):
                    nc.sync.dma_start(
                        out=v_sb[:, kc, :],
                        in_=mmcast(v_d[b, kc * P:(kc + 1) * P, :]),
                    )
                for kc in range(n_kc):
                    for g in range(n_dc // 4):
                        t = tpsp.tile([P, 4, P], mm_dtype, tag="tps")
                        for j in range(4):
                            dc = g * 4 + j
                            nc.tensor.transpose(
                                t[:, j, :],
                                v_sb[:, kc, dc * P:(dc + 1) * P],
                                ident_r[:],
                            )
                        nc.scalar.copy(
                            vt_sb[:, g * 4:(g + 1) * 4, kc * P:(kc + 1) * P], t[:]
                        )

                # Q^T prep for tile qi: load + PE-transpose + ACT copy.
                # Prefetched one tile ahead so the ACT copies never sit on
                # the critical path in front of the S matmuls.
                def qprep(qi, b=b):
                    q0 = qi * P
                    q_nat = qload.tile([P, d], F32, tag="qnat")
                    nc.sync.dma_start(out=q_nat[:], in_=q_d[b, q0:q0 + P, :])
                    qt_sb = qtp.tile([P, n_dc, P], mm_dtype, tag="qt")
                    for g in range(n_dc // 4):
                        t = tpsp.tile([P, 4, P], F32, tag="tps")
                        for j in range(4):
                            dc = g * 4 + j
                            nc.tensor.transpose(
                                t[:, j, :], q_nat[:, dc * P:(dc + 1) * P], ident[:]
                            )
                        nc.scalar.copy(qt_sb[:, g * 4:(g + 1) * 4, :], t[:])
                    return qt_sb

                qt_next = qprep(0)
                for qi in range(n_qt):
                    q0 = qi * P
                    qt_sb = qt_next
                    if qi + 1 < n_qt:
                        qt_next = qprep(qi + 1)

                    # S[q, k] = sum_d Q[q, d] V[k, d], one PSUM bank per 512-wide
                    # chunk; per-chunk max+exp frees each bank early so the
                    # next tile's S matmuls start without waiting for softmax.
                    e_sb = ep.tile([P, lk], F32, tag="e")
                    negmax = statp.tile([P, n_sb], F32, tag="negmax")
                    csum = statp.tile([P, n_sb], F32, tag="csum")
                    for n in range(n_sb):
                        s_ch = spsp.tile([P, NB], F32, tag="sch")
                        for dc in range(n_dc):
                            nc.tensor.matmul(
                                s_ch[:],
                                qt_sb[:, dc, :],
                                vt_sb[:, dc, n * NB:(n + 1) * NB],
                                start=(dc == 0),
                                stop=(dc == n_dc - 1),
                            )
                        nc.vector.reduce_max(
                            out=negmax[:, n:n + 1], in_=s_ch[:],
                            axis=mybir.AxisListType.X, negate=True,
                        )
                        nc.scalar.activation(
                            e_sb[:, n * NB:(n + 1) * NB], s_ch[:],
                            mybir.ActivationFunctionType.Exp,
                            bias=negmax[:, n:n + 1], scale=1.0,
                            accum_out=csum[:, n:n + 1],
                        )

                    # combine chunks exactly: f_n = exp(max_n - M),
                    # total = sum_n f_n * csum_n, g_n = f_n / total
                    negM = statp.tile([P, 1], F32, tag="negM")
                    nc.vector.tensor_reduce(
                        out=negM[:], in_=negmax[:], axis=mybir.AxisListType.X,
                        op=mybir.AluOpType.min,
                    )
                    f = statp.tile([P, n_sb], F32, tag="f")
                    nc.scalar.activation(
                        f[:], negmax[:], mybir.ActivationFunctionType.Exp,
                        bias=negM[:], scale=-1.0,
                    )
                    fc = statp.tile([P, n_sb], F32, tag="fc")
                    stot = statp.tile([P, 1], F32, tag="stot")
                    nc.vector.tensor_mul(fc[:], f[:], csum[:])
                    nc.vector.reduce_sum(
                        out=stot[:], in_=fc[:], axis=mybir.AxisListType.X
                    )
                    rinv = statp.tile([P, 1], F32, tag="rinv")
                    nc.vector.reciprocal(rinv[:], stot[:])
                    g = statp.tile([P, n_sb], F32, tag="g")
                    nc.vector.tensor_scalar_mul(g[:], f[:], rinv[:])

                    # normalize E in place -> final attn row-block
                    for n in range(n_sb):
                        nc.vector.tensor_scalar_mul(
                            e_sb[:, n * NB:(n + 1) * NB],
                            e_sb[:, n * NB:(n + 1) * NB],
                            g[:, n:n + 1],
                        )
                    nc.sync.dma_start(out=attn_d[b, q0:q0 + P, :], in_=e_sb[:])

                    def tail(b=b, q0=q0, e_sb=e_sb, v_sb=v_sb):
                        # A^T tiles: et_sb[p, kc, f] = A[f, kc*P+p]
                        et_sb = etp.tile([P, n_kc, P], mm_dtype, tag="et")
                        for gi in range(n_kc // 4):
                            t = tpsp.tile([P, 4, P], F32, tag="tps")
                            for j in range(4):
                                kc = gi * 4 + j
                                nc.tensor.transpose(
                                    t[:, j, :], e_sb[:, kc * P:(kc + 1) * P],
                                    ident[:],
                                )
                            nc.scalar.copy(et_sb[:, gi * 4:(gi + 1) * 4, :], t[:])

                        # C[q, d] = sum_k A[q, k] V[k, d]
                        c_ps = cpsp.tile([P, d], F32, tag="cps")
                        for kc in range(n_kc):
                            for dh in range(n_cb):
                                nc.tensor.matmul(
                                    c_ps[:, dh * NB:(dh + 1) * NB],
                                    et_sb[:, kc, :],
                                    v_sb[:, kc, dh * NB:(dh + 1) * NB],
                                    start=(kc == 0),
                                    stop=(kc == n_kc - 1),
                                )
                        c_sb = cp.tile([P, d], F32, tag="c")
                        nc.scalar.copy(c_sb[:], c_ps[:])
                        nc.sync.dma_start(out=ctx_d[b, q0:q0 + P, :], in_=c_sb[:])

                    if pending_tail is not None:
                        pending_tail()
                    pending_tail = tail

            if pending_tail is not None:
                pending_tail()
                pending_tail = None

    if split_waits:
        _split_multi_waits(nc)
    nc.finalize()
    return nc


_CACHE: dict = {}


def _get_nc():
    if "nc" not in _CACHE:
        _CACHE["nc"] = build_nc()
    return _CACHE["nc"]


def make_in_maps(query, value):
    query = np.ascontiguousarray(np.asarray(query, dtype=np.float32))
    value = np.ascontiguousarray(np.asarray(value, dtype=np.float32))
    return [
        {
            "query": query[i * BPC:(i + 1) * BPC],
            "value": value[i * BPC:(i + 1) * BPC],
        }
        for i in range(N_CORES)
    ]


def kernel(query, value):
    nc = _get_nc()
    res = run_bass_kernel_spmd(
        nc, make_in_maps(query, value), core_ids=list(range(N_CORES))
    ).results
    context = np.concatenate([r["context"] for r in res], axis=0)
    attn = np.concatenate([r["attn"] for r in res], axis=0)
    return context, attn


# revision 31
# speedup vs baseline: 1.4321x; 1.0273x over previous
"""Dot-product attention (B=16, Lq=Lv=2048, D=1024) on 8 TRN2 NeuronCores.

Data-parallel over the batch dim: core i handles batch elements [2i, 2i+1].
Per batch element, per 128-row q-tile:
  S = Q @ V^T        (fp32r matmuls, contraction d on partitions)
  A = softmax(S)     (DVE row-max, ACT exp with fused row-sum, DVE scale)
  C = A @ V          (fp32r matmuls, contraction k on partitions)
Returns (context, attn) exactly like the reference module.
"""

import sys

import numpy as np

if "/opt/trn_rl_repo" not in sys.path:
    sys.path.insert(0, "/opt/trn_rl_repo")

import concourse.bass as bass
import concourse.mybir as mybir
import concourse.tile as tile
from concourse.bass_utils import run_bass_kernel_spmd
from concourse.masks import make_identity

F32 = mybir.dt.float32
F32R = mybir.dt.float32r
F16 = mybir.dt.float16

B, LQ, LK, D = 16, 2048, 2048, 1024
N_CORES = 8
BPC = B // N_CORES  # batch elements per core
P = 128             # SBUF/PSUM partitions
NB = 512            # one PSUM bank of fp32


def _split_multi_waits(nc):
    """This walrus build allows only one sync-wait command per instruction;
    move extra waits onto standalone EventSemaphore carriers just before."""
    for f in nc.m.functions:
        for blk in f.blocks:
            out = []
            for inst in blk.instructions:
                si = getattr(inst, "sync_info", None)
                if si is not None and si.on_wait is not None and len(si.on_wait) > 1:
                    waits = list(si.on_wait)
                    for w in waits[:-1]:
                        nop = mybir.InstEventSemaphore(
                            name=f"I-{nc.next_id()}", ins=[], outs=[]
                        )
                        nop.engine = inst.engine
                        nop.sync_info = mybir.SyncInfo(on_wait=[w], on_update=[])
                        out.append(nop)
                    inst.sync_info = mybir.SyncInfo(
                        on_wait=[waits[-1]], on_update=list(si.on_update)
                    )
                out.append(inst)
            blk.instructions = out


def build_nc(bpc=BPC, lq=LQ, lk=LK, d=D, mm_dtype=F32R, split_waits=True):
    """Build + compile the single-core Bass program (same program on all cores)."""
    n_qt = lq // P     # q row-blocks per batch element
    n_kc = lk // P     # k chunks (contraction tiles for C; width tiles for S)
    n_dc = d // P      # d chunks (contraction tiles for S)
    n_sb = lk // NB    # PSUM banks per S row-block
    n_cb = d // NB     # PSUM banks per C row-block

    nc = bass.Bass()
    q_d = nc.dram_tensor("query", [bpc, lq, d], F32, kind="ExternalInput")
    v_d = nc.dram_tensor("value", [bpc, lk, d], F32, kind="ExternalInput")
    ctx_d = nc.dram_tensor("context", [bpc, lq, d], F32, kind="ExternalOutput")
    attn_d = nc.dram_tensor("attn", [bpc, lq, lk], F32, kind="ExternalOutput")

    # Tiles consumed by reduced-precision matmuls carry mm_dtype themselves
    # (the BIR verifier requires fp32r consumers to read fp32r-rounded data),
    # so producers (ACT copies / DMA) round on write.
    def mmcast(ap):
        return ap.bitcast(mm_dtype) if mm_dtype != F32 else ap

    with tile.TileContext(nc) as tc:
        with (
            tc.tile_pool(name="const", bufs=1) as constp,
            tc.tile_pool(name="vres", bufs=1) as vres,
            tc.tile_pool(name="qload", bufs=2) as qload,
            tc.tile_pool(name="qt", bufs=2) as qtp,
            tc.tile_pool(name="e", bufs=3) as ep,
            tc.tile_pool(name="ah", bufs=2) as ahp,
            tc.tile_pool(name="et", bufs=2) as etp,
            tc.tile_pool(name="c", bufs=2) as cp,
            tc.tile_pool(name="stats", bufs=3) as statp,
            tc.tile_pool(name="s_ps", bufs=4, space=bass.MemorySpace.PSUM) as spsp,
            tc.tile_pool(name="c_ps", bufs=1, space=bass.MemorySpace.PSUM) as cpsp,
            tc.tile_pool(name="t_ps", bufs=2, space=bass.MemorySpace.PSUM) as tpsp,
        ):
            ident = constp.tile([P, P], F32, tag="ident")
            make_identity(nc, ident[:])
            if mm_dtype != F32:
                ident_r = constp.tile([P, P], mm_dtype, tag="ident_r")
                nc.scalar.copy(ident_r[:], ident[:])
            else:
                ident_r = ident
            ident_h = constp.tile([P, P], F16, tag="ident_h")
            nc.scalar.copy(ident_h[:], ident[:])

            # Software pipeline: each q-tile's "tail" (A^T transposes + C
            # matmuls) is emitted after the NEXT tile's "head" (Q^T + S
            # matmuls).  Engine streams execute in program order, so this
            # gives the PE independent S-work to chew on while the softmax
            # combine chain (DVE/ACT) of the previous tile completes.
            pending_tail = None

            for b in range(bpc):
                if pending_tail is not None:
                    pending_tail()
                    pending_tail = None
                # V resident in two forms:
                #   vt_sb[p, dc, k] = V[k, dc*P + p]   (f32r transposed, for S)
                #   v_h[p, kc, :]   = V[kc*P + p, :]   (fp16 natural, for C)
                # Natural f32r V only passes through a small staging buffer.
                vt_sb = vres.tile([P, n_dc, lk], mm_dtype, tag="vt")
                v_h = vres.tile([P, n_kc, d], F16, tag="vh")
                for kc in range(n_kc):
                    v_stage = qload.tile([P, d], mm_dtype, tag="vstage")
                    nc.sync.dma_start(
                        out=v_stage[:],
                        in_=mmcast(v_d[b, kc * P:(kc + 1) * P, :]),
                    )
                    for g in range(n_dc // 4):
                        t = tpsp.tile([P, 4, P], mm_dtype, tag="tps")
                        for j in range(4):
                            dc = g * 4 + j
                            nc.tensor.transpose(
                                t[:, j, :],
                                v_stage[:, dc * P:(dc + 1) * P],
                                ident_r[:],
                            )
                        nc.scalar.copy(
                            vt_sb[:, g * 4:(g + 1) * 4, kc * P:(kc + 1) * P], t[:]
                        )
                    nc.vector.tensor_copy(v_h[:, kc, :], v_stage[:])

                # Q^T prep for tile qi: load + PE-transpose + ACT copy.
                # Prefetched one tile ahead so the ACT copies never sit on
                # the critical path in front of the S matmuls.
                def qprep(qi, b=b):
                    q0 = qi * P
                    q_nat = qload.tile([P, d], mm_dtype, tag="qnat")
                    nc.sync.dma_start(
                        out=q_nat[:], in_=mmcast(q_d[b, q0:q0 + P, :])
                    )
                    qt_sb = qtp.tile([P, n_dc, P], mm_dtype, tag="qt")
                    for g in range(n_dc // 4):
                        t = tpsp.tile([P, 4, P], mm_dtype, tag="tps")
                        for j in range(4):
                            dc = g * 4 + j
                            nc.tensor.transpose(
                                t[:, j, :], q_nat[:, dc * P:(dc + 1) * P],
                                ident_r[:],
                            )
                        nc.scalar.copy(qt_sb[:, g * 4:(g + 1) * 4, :], t[:])
                    return qt_sb

                qt_next = qprep(0)
                for qi in range(n_qt):
                    q0 = qi * P
                    qt_sb = qt_next
                    if qi + 1 < n_qt:
                        qt_next = qprep(qi + 1)

                    # S[q, k] = sum_d Q[q, d] V[k, d], one PSUM bank per 512-wide
                    # chunk; per-chunk max+exp frees each bank early so the
                    # next tile's S matmuls start without waiting for softmax.
                    e_sb = ep.tile([P, lk], F32, tag="e")
                    negmax = statp.tile([P, n_sb], F32, tag="negmax")
                    csum = statp.tile([P, n_sb], F32, tag="csum")
                    for n in range(n_sb):
                        s_ch = spsp.tile([P, NB], F32, tag="sch")
                        for dc in range(n_dc):
                            nc.tensor.matmul(
                                s_ch[:],
                                qt_sb[:, dc, :],
                                vt_sb[:, dc, n * NB:(n + 1) * NB],
                                start=(dc == 0),
                                stop=(dc == n_dc - 1),
                            )
                        nc.vector.reduce_max(
                            out=negmax[:, n:n + 1], in_=s_ch[:],
                            axis=mybir.AxisListType.X, negate=True,
                        )
                        nc.scalar.activation(
                            e_sb[:, n * NB:(n + 1) * NB], s_ch[:],
                            mybir.ActivationFunctionType.Exp,
                            bias=negmax[:, n:n + 1], scale=1.0,
                            accum_out=csum[:, n:n + 1],
                        )

                    # combine chunks exactly: f_n = exp(max_n - M),
                    # total = sum_n f_n * csum_n, g_n = f_n / total
                    negM = statp.tile([P, 1], F32, tag="negM")
                    nc.vector.tensor_reduce(
                        out=negM[:], in_=negmax[:], axis=mybir.AxisListType.X,
                        op=mybir.AluOpType.min,
                    )
                    f = statp.tile([P, n_sb], F32, tag="f")
                    nc.scalar.activation(
                        f[:], negmax[:], mybir.ActivationFunctionType.Exp,
                        bias=negM[:], scale=-1.0,
                    )
                    fc = statp.tile([P, n_sb], F32, tag="fc")
                    stot = statp.tile([P, 1], F32, tag="stot")
                    nc.vector.tensor_mul(fc[:], f[:], csum[:])
                    nc.vector.reduce_sum(
                        out=stot[:], in_=fc[:], axis=mybir.AxisListType.X
                    )
                    rinv = statp.tile([P, 1], F32, tag="rinv")
                    nc.vector.reciprocal(rinv[:], stot[:])
                    g = statp.tile([P, n_sb], F32, tag="g")
                    nc.vector.tensor_scalar_mul(g[:], f[:], rinv[:])

                    # normalize E in place -> final attn row-block, plus an
                    # fp16 copy of A feeding the (all-fp16) C matmul path
                    a_h = ahp.tile([P, lk], F16, tag="ah")
                    for n in range(n_sb):
                        nc.vector.tensor_scalar_mul(
                            e_sb[:, n * NB:(n + 1) * NB],
                            e_sb[:, n * NB:(n + 1) * NB],
                            g[:, n:n + 1],
                        )
                        nc.vector.tensor_copy(
                            a_h[:, n * NB:(n + 1) * NB],
                            e_sb[:, n * NB:(n + 1) * NB],
                        )
                    nc.sync.dma_start(out=attn_d[b, q0:q0 + P, :], in_=e_sb[:])

                    def tail(b=b, q0=q0, a_h=a_h, v_h=v_h):
                        # A^T tiles (fp16): et_sb[p, kc, f] = A[f, kc*P+p]
                        et_sb = etp.tile([P, n_kc, P], F16, tag="et")
                        for gi in range(n_kc // 4):
                            t = tpsp.tile([P, 4, P], F16, tag="tps")
                            for j in range(4):
                                kc = gi * 4 + j
                                nc.tensor.transpose(
                                    t[:, j, :], a_h[:, kc * P:(kc + 1) * P],
                                    ident_h[:],
                                )
                            nc.scalar.copy(et_sb[:, gi * 4:(gi + 1) * 4, :], t[:])

                        # C[q, d] = sum_k A[q, k] V[k, d]   (fp16 x fp16)
                        c_ps = cpsp.tile([P, d], F32, tag="cps")
                        for kc in range(n_kc):
                            for dh in range(n_cb):
                                nc.tensor.matmul(
                                    c_ps[:, dh * NB:(dh + 1) * NB],
                                    et_sb[:, kc, :],
                                    v_h[:, kc, dh * NB:(dh + 1) * NB],
                                    start=(kc == 0),
                                    stop=(kc == n_kc - 1),
                                )
                        c_sb = cp.tile([P, d], F32, tag="c")
                        nc.scalar.copy(c_sb[:], c_ps[:])
                        nc.sync.dma_start(out=ctx_d[b, q0:q0 + P, :], in_=c_sb[:])

                    if pending_tail is not None:
                        pending_tail()
                    pending_tail = tail

            if pending_tail is not None:
                pending_tail()
                pending_tail = None

    if split_waits:
        _split_multi_waits(nc)
    nc.finalize()
    return nc


_CACHE: dict = {}


def _get_nc():
    if "nc" not in _CACHE:
        _CACHE["nc"] = build_nc()
    return _CACHE["nc"]


def make_in_maps(query, value):
    query = np.ascontiguousarray(np.asarray(query, dtype=np.float32))
    value = np.ascontiguousarray(np.asarray(value, dtype=np.float32))
    return [
        {
            "query": query[i * BPC:(i + 1) * BPC],
            "value": value[i * BPC:(i + 1) * BPC],
        }
        for i in range(N_CORES)
    ]


def kernel(query, value):
    nc = _get_nc()
    res = run_bass_kernel_spmd(
        nc, make_in_maps(query, value), core_ids=list(range(N_CORES))
    ).results
    context = np.concatenate([r["context"] for r in res], axis=0)
    attn = np.concatenate([r["attn"] for r in res], axis=0)
    return context, attn


# revision 33
# speedup vs baseline: 1.5199x; 1.0613x over previous
"""Dot-product attention (B=16, Lq=Lv=2048, D=1024) on 8 TRN2 NeuronCores.

Data-parallel over the batch dim: core i handles batch elements [2i, 2i+1].
Per batch element, per 128-row q-tile:
  S = Q @ V^T        (fp32r matmuls, contraction d on partitions)
  A = softmax(S)     (DVE row-max, ACT exp with fused row-sum, DVE scale)
  C = A @ V          (fp32r matmuls, contraction k on partitions)
Returns (context, attn) exactly like the reference module.
"""

import sys

import numpy as np

if "/opt/trn_rl_repo" not in sys.path:
    sys.path.insert(0, "/opt/trn_rl_repo")

import concourse.bass as bass
import concourse.mybir as mybir
import concourse.tile as tile
from concourse.bass_utils import run_bass_kernel_spmd
from concourse.masks import make_identity

F32 = mybir.dt.float32
F32R = mybir.dt.float32r
F16 = mybir.dt.float16

B, LQ, LK, D = 16, 2048, 2048, 1024
N_CORES = 8
BPC = B // N_CORES  # batch elements per core
P = 128             # SBUF/PSUM partitions
NB = 512            # one PSUM bank of fp32


def _split_multi_waits(nc):
    """This walrus build allows only one sync-wait command per instruction;
    move extra waits onto standalone EventSemaphore carriers just before."""
    for f in nc.m.functions:
        for blk in f.blocks:
            out = []
            for inst in blk.instructions:
                si = getattr(inst, "sync_info", None)
                if si is not None and si.on_wait is not None and len(si.on_wait) > 1:
                    waits = list(si.on_wait)
                    for w in waits[:-1]:
                        nop = mybir.InstEventSemaphore(
                            name=f"I-{nc.next_id()}", ins=[], outs=[]
                        )
                        nop.engine = inst.engine
                        nop.sync_info = mybir.SyncInfo(on_wait=[w], on_update=[])
                        out.append(nop)
                    inst.sync_info = mybir.SyncInfo(
                        on_wait=[waits[-1]], on_update=list(si.on_update)
                    )
                out.append(inst)
            blk.instructions = out


def build_nc(bpc=BPC, lq=LQ, lk=LK, d=D, mm_dtype=F32R, split_waits=True):
    """Build + compile the single-core Bass program (same program on all cores)."""
    n_qt = lq // P     # q row-blocks per batch element
    n_kc = lk // P     # k chunks (contraction tiles for C; width tiles for S)
    n_dc = d // P      # d chunks (contraction tiles for S)
    n_sb = lk // NB    # PSUM banks per S row-block
    n_cb = d // NB     # PSUM banks per C row-block

    nc = bass.Bass()
    q_d = nc.dram_tensor("query", [bpc, lq, d], F32, kind="ExternalInput")
    v_d = nc.dram_tensor("value", [bpc, lk, d], F32, kind="ExternalInput")
    ctx_d = nc.dram_tensor("context", [bpc, lq, d], F32, kind="ExternalOutput")
    attn_d = nc.dram_tensor("attn", [bpc, lq, lk], F32, kind="ExternalOutput")

    # Tiles consumed by reduced-precision matmuls carry mm_dtype themselves
    # (the BIR verifier requires fp32r consumers to read fp32r-rounded data),
    # so producers (ACT copies / DMA) round on write.
    def mmcast(ap):
        return ap.bitcast(mm_dtype) if mm_dtype != F32 else ap

    with tile.TileContext(nc) as tc:
        with (
            tc.tile_pool(name="const", bufs=1) as constp,
            tc.tile_pool(name="vres", bufs=1) as vres,
            tc.tile_pool(name="qload", bufs=2) as qload,
            tc.tile_pool(name="vload", bufs=6) as vload,
            tc.tile_pool(name="qt", bufs=2) as qtp,
            tc.tile_pool(name="e", bufs=3) as ep,
            tc.tile_pool(name="ah", bufs=2) as ahp,
            tc.tile_pool(name="et", bufs=2) as etp,
            tc.tile_pool(name="c", bufs=2) as cp,
            tc.tile_pool(name="stats", bufs=3) as statp,
            tc.tile_pool(name="s_ps", bufs=4, space=bass.MemorySpace.PSUM) as spsp,
            tc.tile_pool(name="c_ps", bufs=1, space=bass.MemorySpace.PSUM) as cpsp,
            tc.tile_pool(name="t_ps", bufs=2, space=bass.MemorySpace.PSUM) as tpsp,
        ):
            ident = constp.tile([P, P], F32, tag="ident")
            make_identity(nc, ident[:])
            if mm_dtype != F32:
                ident_r = constp.tile([P, P], mm_dtype, tag="ident_r")
                nc.scalar.copy(ident_r[:], ident[:])
            else:
                ident_r = ident
            ident_h = constp.tile([P, P], F16, tag="ident_h")
            nc.scalar.copy(ident_h[:], ident[:])

            # Software pipeline: each q-tile's "tail" (A^T transposes + C
            # matmuls) is emitted after the NEXT tile's "head" (Q^T + S
            # matmuls).  Engine streams execute in program order, so this
            # gives the PE independent S-work to chew on while the softmax
            # combine chain (DVE/ACT) of the previous tile completes.
            pending_tail = None

            for b in range(bpc):
                if pending_tail is not None:
                    pending_tail()
                    pending_tail = None
                # V resident in two forms:
                #   vt_sb[p, dc, k] = V[k, dc*P + p]   (f32r transposed, for S)
                #   v_h[p, kc, :]   = V[kc*P + p, :]   (fp16 natural, for C)
                # Natural f32r V only passes through a small staging buffer.
                vt_sb = vres.tile([P, n_dc, lk], mm_dtype, tag="vt")
                v_h = vres.tile([P, n_kc, d], F16, tag="vh")
                for kc in range(n_kc):
                    v_stage = vload.tile([P, d], mm_dtype, tag="vstage")
                    nc.sync.dma_start(
                        out=v_stage[:],
                        in_=mmcast(v_d[b, kc * P:(kc + 1) * P, :]),
                    )
                    for g in range(n_dc // 4):
                        t = tpsp.tile([P, 4, P], mm_dtype, tag="tps")
                        for j in range(4):
                            dc = g * 4 + j
                            nc.tensor.transpose(
                                t[:, j, :],
                                v_stage[:, dc * P:(dc + 1) * P],
                                ident_r[:],
                            )
                        nc.scalar.copy(
                            vt_sb[:, g * 4:(g + 1) * 4, kc * P:(kc + 1) * P], t[:]
                        )
                    nc.vector.tensor_copy(v_h[:, kc, :], v_stage[:])

                # Q^T prep for tile qi: load + PE-transpose + ACT copy.
                # Prefetched one tile ahead so the ACT copies never sit on
                # the critical path in front of the S matmuls.
                def qprep(qi, b=b):
                    q0 = qi * P
                    q_nat = qload.tile([P, d], mm_dtype, tag="qnat")
                    nc.sync.dma_start(
                        out=q_nat[:], in_=mmcast(q_d[b, q0:q0 + P, :])
                    )
                    qt_sb = qtp.tile([P, n_dc, P], mm_dtype, tag="qt")
                    for g in range(n_dc // 4):
                        t = tpsp.tile([P, 4, P], mm_dtype, tag="tps")
                        for j in range(4):
                            dc = g * 4 + j
                            nc.tensor.transpose(
                                t[:, j, :], q_nat[:, dc * P:(dc + 1) * P],
                                ident_r[:],
                            )
                        nc.scalar.copy(qt_sb[:, g * 4:(g + 1) * 4, :], t[:])
                    return qt_sb

                qt_next = qprep(0)
                for qi in range(n_qt):
                    q0 = qi * P
                    qt_sb = qt_next
                    if qi + 1 < n_qt:
                        qt_next = qprep(qi + 1)

                    # S[q, k] = sum_d Q[q, d] V[k, d], one PSUM bank per 512-wide
                    # chunk; per-chunk max+exp frees each bank early so the
                    # next tile's S matmuls start without waiting for softmax.
                    e_sb = ep.tile([P, lk], F32, tag="e")
                    negmax = statp.tile([P, n_sb], F32, tag="negmax")
                    csum = statp.tile([P, n_sb], F32, tag="csum")
                    for n in range(n_sb):
                        s_ch = spsp.tile([P, NB], F32, tag="sch")
                        for dc in range(n_dc):
                            nc.tensor.matmul(
                                s_ch[:],
                                qt_sb[:, dc, :],
                                vt_sb[:, dc, n * NB:(n + 1) * NB],
                                start=(dc == 0),
                                stop=(dc == n_dc - 1),
                            )
                        nc.vector.reduce_max(
                            out=negmax[:, n:n + 1], in_=s_ch[:],
                            axis=mybir.AxisListType.X, negate=True,
                        )
                        nc.scalar.activation(
                            e_sb[:, n * NB:(n + 1) * NB], s_ch[:],
                            mybir.ActivationFunctionType.Exp,
                            bias=negmax[:, n:n + 1], scale=1.0,
                            accum_out=csum[:, n:n + 1],
                        )

                    # combine chunks exactly: f_n = exp(max_n - M),
                    # total = sum_n f_n * csum_n, g_n = f_n / total
                    negM = statp.tile([P, 1], F32, tag="negM")
                    nc.vector.tensor_reduce(
                        out=negM[:], in_=negmax[:], axis=mybir.AxisListType.X,
                        op=mybir.AluOpType.min,
                    )
                    f = statp.tile([P, n_sb], F32, tag="f")
                    nc.scalar.activation(
                        f[:], negmax[:], mybir.ActivationFunctionType.Exp,
                        bias=negM[:], scale=-1.0,
                    )
                    fc = statp.tile([P, n_sb], F32, tag="fc")
                    stot = statp.tile([P, 1], F32, tag="stot")
                    nc.vector.tensor_mul(fc[:], f[:], csum[:])
                    nc.vector.reduce_sum(
                        out=stot[:], in_=fc[:], axis=mybir.AxisListType.X
                    )
                    rinv = statp.tile([P, 1], F32, tag="rinv")
                    nc.vector.reciprocal(rinv[:], stot[:])
                    g = statp.tile([P, n_sb], F32, tag="g")
                    nc.vector.tensor_scalar_mul(g[:], f[:], rinv[:])

                    # normalize E in place -> final attn row-block, plus an
                    # fp16 copy of A feeding the (all-fp16) C matmul path
                    a_h = ahp.tile([P, lk], F16, tag="ah")
                    for n in range(n_sb):
                        nc.vector.tensor_scalar_mul(
                            e_sb[:, n * NB:(n + 1) * NB],
                            e_sb[:, n * NB:(n + 1) * NB],
                            g[:, n:n + 1],
                        )
                        nc.vector.tensor_copy(
                            a_h[:, n * NB:(n + 1) * NB],
                            e_sb[:, n * NB:(n + 1) * NB],
                        )
                    nc.sync.dma_start(out=attn_d[b, q0:q0 + P, :], in_=e_sb[:])

                    def tail(b=b, q0=q0, a_h=a_h, v_h=v_h):
                        # A^T tiles (fp16): et_sb[p, kc, f] = A[f, kc*P+p]
                        et_sb = etp.tile([P, n_kc, P], F16, tag="et")
                        for gi in range(n_kc // 4):
                            t = tpsp.tile([P, 4, P], F16, tag="tps")
                            for j in range(4):
                                kc = gi * 4 + j
                                nc.tensor.transpose(
                                    t[:, j, :], a_h[:, kc * P:(kc + 1) * P],
                                    ident_h[:],
                                )
                            nc.scalar.copy(et_sb[:, gi * 4:(gi + 1) * 4, :], t[:])

                        # C[q, d] = sum_k A[q, k] V[k, d]   (fp16 x fp16)
                        c_ps = cpsp.tile([P, d], F32, tag="cps")
                        for kc in range(n_kc):
                            for dh in range(n_cb):
                                nc.tensor.matmul(
                                    c_ps[:, dh * NB:(dh + 1) * NB],
                                    et_sb[:, kc, :],
                                    v_h[:, kc, dh * NB:(dh + 1) * NB],
                                    start=(kc == 0),
                                    stop=(kc == n_kc - 1),
                                )
                        c_sb = cp.tile([P, d], F32, tag="c")
                        nc.scalar.copy(c_sb[:], c_ps[:])
                        nc.sync.dma_start(out=ctx_d[b, q0:q0 + P, :], in_=c_sb[:])

                    if pending_tail is not None:
                        pending_tail()
                    pending_tail = tail

            if pending_tail is not None:
                pending_tail()
                pending_tail = None

    if split_waits:
        _split_multi_waits(nc)
    nc.finalize()
    return nc


_CACHE: dict = {}


def _get_nc():
    if "nc" not in _CACHE:
        _CACHE["nc"] = build_nc()
    return _CACHE["nc"]


def make_in_maps(query, value):
    query = np.ascontiguousarray(np.asarray(query, dtype=np.float32))
    value = np.ascontiguousarray(np.asarray(value, dtype=np.float32))
    return [
        {
            "query": query[i * BPC:(i + 1) * BPC],
            "value": value[i * BPC:(i + 1) * BPC],
        }
        for i in range(N_CORES)
    ]


def kernel(query, value):
    nc = _get_nc()
    res = run_bass_kernel_spmd(
        nc, make_in_maps(query, value), core_ids=list(range(N_CORES))
    ).results
    context = np.concatenate([r["context"] for r in res], axis=0)
    attn = np.concatenate([r["attn"] for r in res], axis=0)
    return context, attn


# revision 41
# speedup vs baseline: 1.5487x; 1.0190x over previous
"""Dot-product attention (B=16, Lq=Lv=2048, D=1024) on 8 TRN2 NeuronCores.

Data-parallel over the batch dim: core i handles batch elements [2i, 2i+1].
Per batch element, per 128-row q-tile:
  S = Q @ V^T        (fp32r matmuls, contraction d on partitions)
  A = softmax(S)     (DVE row-max, ACT exp with fused row-sum, DVE scale)
  C = A @ V          (fp32r matmuls, contraction k on partitions)
Returns (context, attn) exactly like the reference module.
"""

import sys

import numpy as np

if "/opt/trn_rl_repo" not in sys.path:
    sys.path.insert(0, "/opt/trn_rl_repo")

import concourse.bass as bass
import concourse.mybir as mybir
import concourse.tile as tile
from concourse.bass_utils import run_bass_kernel_spmd
from concourse.masks import make_identity

F32 = mybir.dt.float32
F32R = mybir.dt.float32r
F16 = mybir.dt.float16

B, LQ, LK, D = 16, 2048, 2048, 1024
N_CORES = 8
BPC = B // N_CORES  # batch elements per core
P = 128             # SBUF/PSUM partitions
NB = 512            # one PSUM bank of fp32


def _split_multi_waits(nc):
    """This walrus build allows only one sync-wait command per instruction;
    move extra waits onto standalone EventSemaphore carriers just before."""
    for f in nc.m.functions:
        for blk in f.blocks:
            out = []
            for inst in blk.instructions:
                si = getattr(inst, "sync_info", None)
                if si is not None and si.on_wait is not None and len(si.on_wait) > 1:
                    waits = list(si.on_wait)
                    for w in waits[:-1]:
                        nop = mybir.InstEventSemaphore(
                            name=f"I-{nc.next_id()}", ins=[], outs=[]
                        )
                        nop.engine = inst.engine
                        nop.sync_info = mybir.SyncInfo(on_wait=[w], on_update=[])
                        out.append(nop)
                    inst.sync_info = mybir.SyncInfo(
                        on_wait=[waits[-1]], on_update=list(si.on_update)
                    )
                out.append(inst)
            blk.instructions = out


def build_nc(bpc=BPC, lq=LQ, lk=LK, d=D, mm_dtype=F32R, split_waits=True):
    """Build + compile the single-core Bass program (same program on all cores)."""
    n_qt = lq // P     # q row-blocks per batch element
    n_kc = lk // P     # k chunks (contraction tiles for C; width tiles for S)
    n_dc = d // P      # d chunks (contraction tiles for S)
    n_sb = lk // NB    # PSUM banks per S row-block
    n_cb = d // NB     # PSUM banks per C row-block

    nc = bass.Bass()
    q_d = nc.dram_tensor("query", [bpc, lq, d], F32, kind="ExternalInput")
    v_d = nc.dram_tensor("value", [bpc, lk, d], F32, kind="ExternalInput")
    ctx_d = nc.dram_tensor("context", [bpc, lq, d], F32, kind="ExternalOutput")
    attn_d = nc.dram_tensor("attn", [bpc, lq, lk], F32, kind="ExternalOutput")

    # Tiles consumed by reduced-precision matmuls carry mm_dtype themselves
    # (the BIR verifier requires fp32r consumers to read fp32r-rounded data),
    # so producers (ACT copies / DMA) round on write.
    def mmcast(ap):
        return ap.bitcast(mm_dtype) if mm_dtype != F32 else ap

    with tile.TileContext(nc) as tc:
        with (
            tc.tile_pool(name="const", bufs=1) as constp,
            tc.tile_pool(name="vres", bufs=1) as vres,
            tc.tile_pool(name="qload", bufs=2) as qload,
            tc.tile_pool(name="vload", bufs=6) as vload,
            tc.tile_pool(name="qt", bufs=2) as qtp,
            tc.tile_pool(name="e", bufs=3) as ep,
            tc.tile_pool(name="ah", bufs=2) as ahp,
            tc.tile_pool(name="et", bufs=2) as etp,
            tc.tile_pool(name="c", bufs=2) as cp,
            tc.tile_pool(name="stats", bufs=3) as statp,
            tc.tile_pool(name="s_ps", bufs=4, space=bass.MemorySpace.PSUM) as spsp,
            tc.tile_pool(name="c_ps", bufs=1, space=bass.MemorySpace.PSUM) as cpsp,
            tc.tile_pool(name="t_ps", bufs=2, space=bass.MemorySpace.PSUM) as tpsp,
        ):
            ident = constp.tile([P, P], F32, tag="ident")
            make_identity(nc, ident[:])
            if mm_dtype != F32:
                ident_r = constp.tile([P, P], mm_dtype, tag="ident_r")
                nc.scalar.copy(ident_r[:], ident[:])
            else:
                ident_r = ident
            ident_h = constp.tile([P, P], F16, tag="ident_h")
            nc.scalar.copy(ident_h[:], ident[:])

            # Software pipeline: each q-tile's "tail" (A^T transposes + C
            # matmuls) is emitted after the NEXT tile's "head" (Q^T + S
            # matmuls).  Engine streams execute in program order, so this
            # gives the PE independent S-work to chew on while the softmax
            # combine chain (DVE/ACT) of the previous tile completes.
            pending_tail = None

            for b in range(bpc):
                if pending_tail is not None:
                    pending_tail()
                    pending_tail = None

                # Q^T prep for tile qi: load + PE-transpose + ACT copy.
                # Prefetched one tile ahead so the ACT copies never sit on
                # the critical path in front of the S matmuls.
                def qprep(qi, b=b):
                    q0 = qi * P
                    q_nat = qload.tile([P, d], mm_dtype, tag="qnat")
                    nc.sync.dma_start(
                        out=q_nat[:], in_=mmcast(q_d[b, q0:q0 + P, :])
                    )
                    qt_sb = qtp.tile([P, n_dc, P], mm_dtype, tag="qt")
                    for g in range(n_dc // 4):
                        t = tpsp.tile([P, 4, P], mm_dtype, tag="tps")
                        for j in range(4):
                            dc = g * 4 + j
                            nc.tensor.transpose(
                                t[:, j, :], q_nat[:, dc * P:(dc + 1) * P],
                                ident_r[:],
                            )
                        nc.scalar.copy(qt_sb[:, g * 4:(g + 1) * 4, :], t[:])
                    return qt_sb

                qt_next = qprep(0)

                # V resident in two forms:
                #   vt_sb[p, dc, k] = V[k, dc*P + p]   (f32r transposed, for S)
                #   v_h[p, kc, :]   = V[kc*P + p, :]   (fp16 natural, for C)
                # Natural f32r V only passes through a small staging buffer.
                vt_sb = vres.tile([P, n_dc, lk], mm_dtype, tag="vt")
                v_h = vres.tile([P, n_kc, d], F16, tag="vh")
                for kc in range(n_kc):
                    v_stage = vload.tile([P, d], mm_dtype, tag="vstage")
                    nc.sync.dma_start(
                        out=v_stage[:],
                        in_=mmcast(v_d[b, kc * P:(kc + 1) * P, :]),
                    )
                    for g in range(n_dc // 4):
                        t = tpsp.tile([P, 4, P], mm_dtype, tag="tps")
                        for j in range(4):
                            dc = g * 4 + j
                            nc.tensor.transpose(
                                t[:, j, :],
                                v_stage[:, dc * P:(dc + 1) * P],
                                ident_r[:],
                            )
                        nc.scalar.copy(
                            vt_sb[:, g * 4:(g + 1) * 4, kc * P:(kc + 1) * P], t[:]
                        )
                    nc.vector.tensor_copy(v_h[:, kc, :], v_stage[:])

                for qi in range(n_qt):
                    q0 = qi * P
                    qt_sb = qt_next
                    if qi + 1 < n_qt:
                        qt_next = qprep(qi + 1)

                    # S[q, k] = sum_d Q[q, d] V[k, d], one PSUM bank per 512-wide
                    # chunk; per-chunk max+exp frees each bank early so the
                    # next tile's S matmuls start without waiting for softmax.
                    e_sb = ep.tile([P, lk], F32, tag="e")
                    negmax = statp.tile([P, n_sb], F32, tag="negmax")
                    csum = statp.tile([P, n_sb], F32, tag="csum")
                    # two chunks per weight load (dc-inner pairs) so walrus's
                    # ldw-opt can drop every other LDWEIGHTS
                    for half in range(n_sb // 2):
                        s_ch0 = spsp.tile([P, NB], F32, tag="sch")
                        s_ch1 = spsp.tile([P, NB], F32, tag="sch")
                        s_chs = [s_ch0, s_ch1]
                        for dc in range(n_dc):
                            for m, s_ch in enumerate(s_chs):
                                n = half * 2 + m
                                nc.tensor.matmul(
                                    s_ch[:],
                                    qt_sb[:, dc, :],
                                    vt_sb[:, dc, n * NB:(n + 1) * NB],
                                    start=(dc == 0),
                                    stop=(dc == n_dc - 1),
                                )
                        for m, s_ch in enumerate(s_chs):
                            n = half * 2 + m
                            nc.vector.reduce_max(
                                out=negmax[:, n:n + 1], in_=s_ch[:],
                                axis=mybir.AxisListType.X, negate=True,
                            )
                            nc.scalar.activation(
                                e_sb[:, n * NB:(n + 1) * NB], s_ch[:],
                                mybir.ActivationFunctionType.Exp,
                                bias=negmax[:, n:n + 1], scale=1.0,
                                accum_out=csum[:, n:n + 1],
                            )

                    # combine chunks exactly: f_n = exp(max_n - M),
                    # total = sum_n f_n * csum_n, g_n = f_n / total
                    negM = statp.tile([P, 1], F32, tag="negM")
                    nc.vector.tensor_reduce(
                        out=negM[:], in_=negmax[:], axis=mybir.AxisListType.X,
                        op=mybir.AluOpType.min,
                    )
                    f = statp.tile([P, n_sb], F32, tag="f")
                    nc.scalar.activation(
                        f[:], negmax[:], mybir.ActivationFunctionType.Exp,
                        bias=negM[:], scale=-1.0,
                    )
                    fc = statp.tile([P, n_sb], F32, tag="fc")
                    stot = statp.tile([P, 1], F32, tag="stot")
                    nc.vector.tensor_mul(fc[:], f[:], csum[:])
                    nc.vector.reduce_sum(
                        out=stot[:], in_=fc[:], axis=mybir.AxisListType.X
                    )
                    rinv = statp.tile([P, 1], F32, tag="rinv")
                    nc.vector.reciprocal(rinv[:], stot[:])
                    g = statp.tile([P, n_sb], F32, tag="g")
                    nc.vector.tensor_scalar_mul(g[:], f[:], rinv[:])

                    # normalize E in place -> final attn row-block, plus an
                    # fp16 copy of A feeding the (all-fp16) C matmul path
                    a_h = ahp.tile([P, lk], F16, tag="ah")
                    for n in range(n_sb):
                        nc.vector.tensor_scalar_mul(
                            e_sb[:, n * NB:(n + 1) * NB],
                            e_sb[:, n * NB:(n + 1) * NB],
                            g[:, n:n + 1],
                        )
                        nc.vector.tensor_copy(
                            a_h[:, n * NB:(n + 1) * NB],
                            e_sb[:, n * NB:(n + 1) * NB],
                        )
                    nc.sync.dma_start(out=attn_d[b, q0:q0 + P, :], in_=e_sb[:])

                    def tail(b=b, q0=q0, a_h=a_h, v_h=v_h):
                        # A^T tiles (fp16): et_sb[p, kc, f] = A[f, kc*P+p]
                        et_sb = etp.tile([P, n_kc, P], F16, tag="et")
                        for gi in range(n_kc // 4):
                            t = tpsp.tile([P, 4, P], F16, tag="tps")
                            for j in range(4):
                                kc = gi * 4 + j
                                nc.tensor.transpose(
                                    t[:, j, :], a_h[:, kc * P:(kc + 1) * P],
                                    ident_h[:],
                                )
                            nc.scalar.copy(et_sb[:, gi * 4:(gi + 1) * 4, :], t[:])

                        # C[q, d] = sum_k A[q, k] V[k, d]   (fp16 x fp16)
                        c_ps = cpsp.tile([P, d], F32, tag="cps")
                        for kc in range(n_kc):
                            for dh in range(n_cb):
                                nc.tensor.matmul(
                                    c_ps[:, dh * NB:(dh + 1) * NB],
                                    et_sb[:, kc, :],
                                    v_h[:, kc, dh * NB:(dh + 1) * NB],
                                    start=(kc == 0),
                                    stop=(kc == n_kc - 1),
                                )
                        c_sb = cp.tile([P, d], F32, tag="c")
                        nc.scalar.copy(c_sb[:], c_ps[:])
                        nc.sync.dma_start(out=ctx_d[b, q0:q0 + P, :], in_=c_sb[:])

                    if pending_tail is not None:
                        pending_tail()
                    pending_tail = tail

            if pending_tail is not None:
                pending_tail()
                pending_tail = None

    if split_waits:
        _split_multi_waits(nc)
    nc.finalize()
    return nc


_CACHE: dict = {}

LDW_OPT = False


def _enable_ldw_opt():
    """Turn on walrus's redundant-LDWEIGHTS elimination (the default bass
    compile pins it off); back-to-back matmuls sharing a stationary operand
    then skip the reload."""
    if _CACHE.get("ldw_patched"):
        return
    import concourse.bass_utils as bu

    orig = bu.run_command

    def patched(cmd, **kw):
        cmd = [
            c.replace("--enable-ldw-opt=false", "--enable-ldw-opt=true")
            if isinstance(c, str)
            else c
            for c in cmd
        ]
        return orig(cmd, **kw)

    bu.run_command = patched
    _CACHE["ldw_patched"] = True


def _get_nc():
    if "nc" not in _CACHE:
        if LDW_OPT:
            _enable_ldw_opt()
        _CACHE["nc"] = build_nc()
    return _CACHE["nc"]


def make_in_maps(query, value):
    query = np.ascontiguousarray(np.asarray(query, dtype=np.float32))
    value = np.ascontiguousarray(np.asarray(value, dtype=np.float32))
    return [
        {
            "query": query[i * BPC:(i + 1) * BPC],
            "value": value[i * BPC:(i + 1) * BPC],
        }
        for i in range(N_CORES)
    ]


def kernel(query, value):
    nc = _get_nc()
    res = run_bass_kernel_spmd(
        nc, make_in_maps(query, value), core_ids=list(range(N_CORES))
    ).results
    context = np.concatenate([r["context"] for r in res], axis=0)
    attn = np.concatenate([r["attn"] for r in res], axis=0)
    return context, attn
